# revision 32
# baseline (speedup 1.0000x reference)
"""Conv1d (B=32, C_in=C_out=64, L=16384, K=3, VALID) on 8 trn2 cores.

Strategy: data-parallel over batch (4 batches/core), POLYPHASE compute.
The host deinterleaves each batch's length axis into even/odd streams,
giving a [128 = (parity x 64 ch), L/2] SBUF view. One rhs column then
carries TWO input samples, and two PSUM-accumulated matmuls against
dense-ish [128,128] weights (A for column t, B for column t+1) produce
TWO output samples per column:
  out[(p=0)*64+co, t] = y[co, 2t]   = w0 xe[t] + w1 xo[t] + w2 xe[t+1]
  out[(p=1)*64+co, t] = y[co, 2t+1] = w0 xo[t] + w1 xe[t+1] + w2 xo[t+1]
  A[e*64+ci, p*64+co]: (0,0)=w0T (1,0)=w1T (1,1)=w0T;  (0,1)=0
  B[e*64+ci, p*64+co]: (0,0)=w2T (0,1)=w1T (1,1)=w2T;  (1,0)=0
That is 1.0 PE cycle per output sample per batch (75% array util) vs
1.5 (50%) for the tap-per-matmul block-diagonal scheme — the matmul
stream drops from ~43.5us to ~29us and the kernel becomes DMA-paced.
fp32 PSUM accumulation; f16 I/O halves HBM traffic (memory roofline).
Bias is fused into the PSUM->SBUF copy, split across ACT and DVE.
Host re-interleaves the [128, L/2] output back to [64, LOUT].

Queue/pipeline findings baked in (perfetto-driven, see session notes):
- ~7us NEFF preamble + ~3.3us teardown are fixed; DMA+PE clocks ramp
  (HAM) over the first ~10us — warm-up matmuls ride it out.
- Inputs must all stay on Sync HWDGE (SWDGE inputs starve the PE);
  weights ride SWDGE (idle early) so Sync's first descriptor is input
  chunk 0; Scalar HWDGE (q10) is ~1KB-packet slow — never used.
- Outputs ride SWDGE except the last SYNC_TAIL chunks on Sync (idle by
  then, fast drain); tail chunks shrink so the last compute->DMA->drain
  hop is short.
- Input DMAs are software-pipelined IBUFS-1 chunks ahead (the single
  biggest scheduling win, ~4us): all inputs hit the Sync queue early so
  tail outputs never head-of-line block one. Emitting inputs further
  ahead than IBUFS-1 would race the tile pool's WAR tracking.
- The fine RAMP chunks are load-bearing even though the PE no longer
  paces the kernel: early compute opens the output stream early, and
  the output side needs the whole window to fit 8.39MB at its ~50%
  engine share. RAMP=[2048]/no-ramp measured 3-9us slower.
- Measured good-phase exec ~56.5us: ~7us NEFF preamble + ~45us DMA
  window (16.85MB, ~420 GB/s engine aggregate mid-run, slow ~8us
  clock ramp) + ~3.5us barrier teardown. The chip oscillates into a
  throttled phase (util limit ~0.67) where the same NEFF runs 61-66us.
"""

import os

import numpy as np

from concourse import bacc, bass, mybir, tile
from concourse.bass_utils import run_bass_kernel_spmd

B, C, L, K = 32, 64, 16384, 3
LOUT = L - K + 1  # 16382
NCORES = 8
BPC = B // NCORES  # 4 batches per core
P = 128  # partitions (2 x C)
NJ = 512  # PSUM inner chunk (one fp32 bank; ISA max matmul free dim)
TH = L // 2  # 8192 deinterleaved columns
TOUT = LOUT // 2  # 8191 output column-pairs

F32 = mybir.dt.float32

# precision mode: f16 I/O (default, ~3e-4 rel err) or f32
MODE = os.environ.get("CONV_MODE", "f16")
CH = int(os.environ.get("CONV_CH", "4096"))  # T-cols per chunk (2x samples)
IBUFS = int(os.environ.get("CONV_IBUFS", "8"))
OBUFS = int(os.environ.get("CONV_OBUFS", "6"))
WARMUP = int(os.environ.get("CONV_WARMUP", "4"))
# dummy matmuls after the real work: hold the HAM clock up while the
# output tail drains (an idle PE down-clocks the core and the trailing
# DMA issues/transfers crawl)
WARMDOWN = int(os.environ.get("CONV_WARMDOWN", "16"))
SYNC_TAIL = int(os.environ.get("CONV_SYNC_TAIL", "4"))
WQ = os.environ.get("CONV_WQ", "gpsimd")
RAMP = [int(v) for v in os.environ.get("CONV_RAMP", "512,1024").split(",") if v]
TAIL = [int(v) for v in os.environ.get("CONV_TAIL", "512,256").split(",") if v]
# output DMA slice granularity (T-cols): big slices = 8KB rows (best
# per-engine rate) for leading batches, finer slices for the last batch
# so trailing writes don't leave the DMA engines under-fed at the end.
OSPLIT = int(os.environ.get("CONV_OSPLIT", "0"))  # 0 = per input chunk
OSPLIT_LAST = int(os.environ.get("CONV_OSPLIT_LAST", "2048"))

_NC_CACHE = []


def _io_dtypes():
    if MODE == "f16":
        return mybir.dt.float16, mybir.dt.float16, np.float16
    return F32, F32, np.float32


def _chunk_lists():
    """Per-batch T-column chunk lists. Batch 0 ramps up (DMA/PE clocks
    still ramping), the last batch ramps down (short tail)."""
    lists = {}
    for b in range(BPC):
        pre = RAMP if b == 0 else []
        post = TAIL if b == BPC - 1 else []
        rest = TOUT - sum(pre) - sum(post)
        body = [CH] * (rest // CH)
        last = rest - sum(body)
        lists[b] = pre + body + ([last] if last else []) + post
        assert sum(lists[b]) == TOUT and all(n > 0 for n in lists[b])
    return lists


def _build_nc():
    FIN, FOUT, _ = _io_dtypes()
    nc = bacc.Bacc("TRN2",
                   target_bir_lowering=bool(int(
                       os.environ.get("CONV_BIRLOW", "0"))),
                   debug=False, num_devices=NCORES)

    xd = nc.dram_tensor("xd", [BPC, P, TH], FIN, kind="ExternalInput")
    wT = nc.dram_tensor("wT", [P, 2, P], FIN, kind="ExternalInput")
    b2 = nc.dram_tensor("b2", [P, 1], F32, kind="ExternalInput")
    yd = nc.dram_tensor("yd", [BPC, P, TOUT], FOUT, kind="ExternalOutput")

    chunk_lists = _chunk_lists()
    nchunks = sum(len(v) for v in chunk_lists.values())

    with tile.TileContext(nc) as tc:
        with (
            tc.tile_pool(name="const", bufs=1) as const_pool,
            tc.tile_pool(name="inp", bufs=IBUFS) as inp_pool,
            tc.tile_pool(name="outp", bufs=OBUFS) as outp_pool,
            tc.tile_pool(name="psum", bufs=8, space=bass.MemorySpace.PSUM)
            as psum_pool,
        ):
            weng = nc.gpsimd if WQ == "gpsimd" else nc.sync
            w = const_pool.tile([P, 2, P], FIN)
            weng.dma_start(out=w[:], in_=wT[:])
            bias = const_pool.tile([P, 1], F32)
            weng.dma_start(out=bias[:], in_=b2[:])

            # HAM warm-up: dummy matmuls on zeroed SBUF while the first
            # input DMA is in flight, so clocks are ramped when real work
            # arrives. memset on GpSimd so this isn't gated on DVE start.
            if WARMUP or WARMDOWN:
                wz = const_pool.tile([P, 512], FIN)
                nc.gpsimd.memset(wz[:], 0.0)
                for i in range(WARMUP):
                    wp = psum_pool.tile([P, NJ], F32, tag="acc",
                                        name=f"warm{i}")
                    nc.tensor.matmul(wp[:], wz[:, :P], wz[:],
                                     start=True, stop=True)

            # flat schedule with per-chunk output slice bounds
            sched = []
            nslices = 0
            for b in range(BPC):
                t0 = 0
                for nT in chunk_lists[b]:
                    osz = OSPLIT_LAST if b == BPC - 1 else OSPLIT
                    osz = osz or nT
                    bounds = []
                    o0 = 0
                    for j0 in range(0, nT, NJ):
                        je = min(j0 + NJ, nT)
                        if je - o0 >= osz or je == nT:
                            bounds.append((o0, je))
                            o0 = je
                    sched.append((b, t0, nT, bounds))
                    nslices += len(bounds)
                    t0 += nT

            # software-pipelined input prefetch: issue input DMA for chunk
            # c+LOOK during chunk c's section (LOOK = IBUFS-1 keeps the
            # pool's WAR deps ordered). All inputs hit the Sync queue early,
            # so tail outputs on Sync never head-of-line block an input.
            # rhs needs one halo column (t0+nT+1 <= TOUT+1 <= TH).
            look = IBUFS - 1
            in_tiles = {}

            def issue_in(idx):
                if idx >= len(sched):
                    return
                bb, tt0, nnT, _ = sched[idx]
                it = inp_pool.tile([P, CH + 1], FIN, tag="in")
                nc.sync.dma_start(out=it[:, :nnT + 1],
                                  in_=xd[bb, :, tt0:tt0 + nnT + 1])
                in_tiles[idx] = it

            for idx in range(min(look, len(sched))):
                issue_in(idx)

            si = 0
            for ci, (b, t0, nT, bounds) in enumerate(sched):
                issue_in(ci + look)
                it = in_tiles.pop(ci)
                ot = outp_pool.tile([P, CH], FOUT, tag="out")
                bi = 0
                for j0 in range(0, nT, NJ):
                    nj = min(NJ, nT - j0)
                    pt = psum_pool.tile([P, NJ], F32, tag="acc")
                    nc.tensor.matmul(pt[:, :nj], w[:, 0, :],
                                     it[:, j0:j0 + nj],
                                     start=True, stop=False)
                    nc.tensor.matmul(pt[:, :nj], w[:, 1, :],
                                     it[:, j0 + 1:j0 + 1 + nj],
                                     start=False, stop=True)
                    # psum -> sbuf with fused bias add, split across
                    # ACT and DVE so the bank frees twice as fast
                    h = nj // 2
                    nc.scalar.add(ot[:, j0:j0 + h], pt[:, :h],
                                  add=bias[:, 0:1])
                    nc.vector.tensor_scalar_add(ot[:, j0 + h:j0 + nj],
                                                pt[:, h:nj],
                                                bias[:, 0:1])
                    if bi < len(bounds) and j0 + nj == bounds[bi][1]:
                        o0, oe = bounds[bi]
                        # tail slices alternate Sync/GpSimd so two engines
                        # issue in parallel (issue costs ~0.64us each); the
                        # very last slice rides Sync for a fast final drain.
                        # Sync outputs sit after all inputs in its queue, so
                        # no input is head-of-line blocked.
                        if si >= nslices - SYNC_TAIL:
                            oeng = nc.sync if (nslices - si) % 2 == 1 \
                                else nc.gpsimd
                        else:
                            oeng = nc.gpsimd
                        oeng.dma_start(out=yd[b, :, t0 + o0:t0 + oe],
                                       in_=ot[:, o0:oe])
                        bi += 1
                        si += 1

            if WARMDOWN:
                for i in range(WARMDOWN):
                    wp = psum_pool.tile([P, NJ], F32, tag="acc",
                                        name=f"wd{i}")
                    nc.tensor.matmul(wp[:], wz[:, :P], wz[:],
                                     start=True, stop=True)

    nc.compile()
    return nc


def _get_nc():
    if not _NC_CACHE:
        _NC_CACHE.append(_build_nc())
    return _NC_CACHE[0]


def _prep_weights(weight, bias, np_in):
    w0T, w1T, w2T = (np.ascontiguousarray(weight[:, :, k].T)
                     for k in range(K))
    A = np.zeros((P, P), np.float32)
    Bm = np.zeros((P, P), np.float32)
    A[0:C, 0:C] = w0T
    A[C:P, 0:C] = w1T
    A[C:P, C:P] = w0T
    Bm[0:C, 0:C] = w2T
    Bm[0:C, C:P] = w1T
    Bm[C:P, C:P] = w2T
    wT = np.stack([A, Bm], axis=1).astype(np_in)  # [P, 2, P]
    b2 = np.concatenate([bias, bias]).reshape(P, 1).astype(np.float32)
    return wT, b2


def kernel(x, weight, bias, _want_results=False, **run_kwargs):
    x = np.asarray(x, np.float32)
    weight = np.asarray(weight, np.float32)
    bias = np.asarray(bias, np.float32)
    _, _, np_in = _io_dtypes()
    nc = _get_nc()
    wT, b2 = _prep_weights(weight, bias, np_in)
    in_maps = []
    for i in range(NCORES):
        xs = x[BPC * i:BPC * (i + 1)]  # [BPC, C, L]
        # deinterleave: partition row e*64+ci holds x[ci, e::2]
        xdi = np.ascontiguousarray(
            xs.reshape(BPC, C, TH, 2).transpose(0, 3, 1, 2)
        ).reshape(BPC, P, TH).astype(np_in, copy=False)
        in_maps.append({"xd": xdi, "wT": wT, "b2": b2})
    res = run_bass_kernel_spmd(nc, in_maps, list(range(NCORES)), **run_kwargs)
    outs = []
    for i in range(NCORES):
        ydi = res.results[i]["yd"].astype(np.float32)  # [BPC, P, TOUT]
        # re-interleave: y[co, 2t+p] = yd[p*64+co, t]
        y = ydi.reshape(BPC, 2, C, TOUT).transpose(0, 2, 3, 1).reshape(
            BPC, C, LOUT)
        outs.append(y)
    out = np.ascontiguousarray(np.concatenate(outs, axis=0))
    if _want_results:
        return out, res
    return out
